# revision 8
# baseline (speedup 1.0000x reference)
"""Distributed Trainium2 kernel for LayerNorm + multi-head self-attention + out-proj.

Reference model (dims hardcoded):
  x [2, 2048, 1024] -> LayerNorm(gamma, beta) -> QKV (w_qkv [1024, 3072])
  -> 16-head attention (d_head 64, scale 1/8) -> out proj (w_out [1024,1024] + b_out)

Sharding (8 NeuronCores): pure head tensor-parallelism. Core g owns global heads
{2g, 2g+1} and processes BOTH batches (tokens flattened to [4096, 1024]).
LayerNorm is computed redundantly per core (cheap). After attention, a
per-head AllToAll redistributes the (normalized) attention output so core g
holds all 1024 inner dims for flat token rows [g*512, (g+1)*512); the out
projection is then local and the host just concatenates the 8 slices.

Numerics: fp32 storage, fp32r (TF32-like, ~1e-4) TensorEngine compute with
fp32 PSUM accumulation. Softmax skips the max-subtraction pass (scores are
~N(0,1) for this model family; exp stays in fp32 range) which lets the
softmax denominator come for free from a ones-augmented column in the PV
matmul's stationary operand.
"""
import numpy as np

import concourse.bass as bass
import concourse.mybir as mybir
import concourse.tile as tile
from concourse import bacc
from concourse.bass_utils import run_bass_kernel_spmd

F32 = mybir.dt.float32
F32R = mybir.dt.float32r
AF = mybir.ActivationFunctionType
OP = mybir.AluOpType

B = 2
N = 2048
D = 1024
HEADS = 16
DH = 64
SCALE = 0.125
EPS = 1e-5

NT = B * N              # 4096 flat tokens
P = 128                 # partition tile
NTILES = NT // P        # 32 token tiles
NBLK = NT // 512        # 8 token blocks of 512
DC = D // P             # 8 contraction chunks
H_LOC = 2               # heads per core
QKV_COLS = 3 * H_LOC * DH   # 384 local qkv cols
TOK_OUT = NT // 8       # 512 output rows per core


def _build():
    nc = bacc.Bacc("TRN2", target_bir_lowering=False, debug=False, num_devices=8)

    x_ext = nc.dram_tensor("x", [NT, D], F32, kind="ExternalInput")
    wqkv_ext = nc.dram_tensor("wqkv", [D, QKV_COLS], F32, kind="ExternalInput")
    bqkv_ext = nc.dram_tensor("bqkv", [1, QKV_COLS], F32, kind="ExternalInput")
    wout_ext = nc.dram_tensor("wout", [D, D], F32, kind="ExternalInput")
    bout_ext = nc.dram_tensor("bout", [1, D], F32, kind="ExternalInput")
    id_ext = nc.dram_tensor("ident", [P, P], F32, kind="ExternalInput")
    out_ext = nc.dram_tensor("out", [TOK_OUT, D], F32, kind="ExternalOutput")

    with tile.TileContext(nc) as tc:
        with tc.tile_pool(name="persist", bufs=1) as pp, \
             tc.tile_pool(name="xs", bufs=3) as xsp, \
             tc.tile_pool(name="xnt", bufs=16) as xntp, \
             tc.tile_pool(name="es", bufs=3) as esp, \
             tc.tile_pool(name="sans", bufs=4) as sanp, \
             tc.tile_pool(name="small", bufs=4) as smp, \
             tc.tile_pool(name="dram", bufs=1, space="DRAM") as dram, \
             tc.tile_pool(name="ps_s", bufs=2, space="PSUM") as ps_s, \
             tc.tile_pool(name="ps_sa", bufs=1, space="PSUM") as ps_sa, \
             tc.tile_pool(name="ps_m", bufs=2, space="PSUM") as ps_m:

            # ---- constants / weights -------------------------------------
            ident = pp.tile([P, P], F32R, tag="ident")
            nc.gpsimd.dma_start(ident[:], id_ext.ap())

            ones512_32 = pp.tile([1, 512], F32, tag="ones512_32")
            nc.vector.memset(ones512_32[:], 1.0)
            ones512 = pp.tile([1, 512], F32R, tag="ones512")
            nc.vector.tensor_copy(ones512[:], ones512_32[:])
            # [1, 64] / [1, 128] ones for K=1 broadcast matmuls
            ones_col64 = pp.tile([1, 64], F32R, tag="ones_col64")
            nc.vector.tensor_copy(ones_col64[:], ones512_32[:, 0:64])
            ones_col128 = pp.tile([1, 128], F32R, tag="ones_col128")
            nc.vector.tensor_copy(ones_col128[:], ones512_32[:, 0:128])
            onesp_32 = pp.tile([P, 1], F32, tag="onesp_32")
            nc.vector.memset(onesp_32[:], 1.0)
            onesp = pp.tile([P, 1], F32R, tag="onesp")
            nc.vector.tensor_copy(onesp[:], onesp_32[:])
            epsp = pp.tile([P, 1], F32, tag="epsp")
            nc.vector.memset(epsp[:], EPS)

            wqkv = []
            for c in range(DC):
                t = pp.tile([P, QKV_COLS], F32R, tag=f"wqkv{c}")
                nc.gpsimd.dma_start(t[:], wqkv_ext.ap()[c * P:(c + 1) * P, :])
                wqkv.append(t)
            bqkv = pp.tile([1, QKV_COLS], F32R, tag="bqkv")
            nc.gpsimd.dma_start(bqkv[:], bqkv_ext.ap())
            bout = pp.tile([1, D], F32R, tag="bout")
            nc.gpsimd.dma_start(bout[:], bout_ext.ap())

            # persistent activation tensors
            qT = pp.tile([P, NT], F32R, tag="qT")    # parts h*64.. = head h
            kT = pp.tile([P, NT], F32R, tag="kT")
            # v_aug: per token tile [128, 130]: [v_h0(64) | 1 | v_h1(64) | 1]
            vaug = pp.tile([P, NTILES * 130], F32R, tag="vaug")

            # a2a buffers (per local head)
            a2a_in = [dram.tile([8, DH, 512], F32, name=f"a2a_in{h}", tag=f"a2a_in{h}")
                      for h in range(H_LOC)]
            a2a_out = [dram.tile([8, DH, 512], F32, name=f"a2a_out{h}", tag=f"a2a_out{h}")
                       for h in range(H_LOC)]

            # ---- phase 1+2: LayerNorm -> xn^T, then QKV per 512-block ----
            xnt = {}   # (blk, c) -> [128, 512] f32r tile (xn^T chunk)

            def ln_tile(i):
                """LayerNorm token tile i -> write xnT chunks."""
                xt = xsp.tile([P, D], F32, tag="x")
                nc.sync.dma_start(xt[:], x_ext.ap()[i * P:(i + 1) * P, :])
                stats = smp.tile([P, 2, 6], F32, tag="stats")
                nc.vector.bn_stats(stats[:, 0, :], xt[:, 0:512])
                nc.vector.bn_stats(stats[:, 1, :], xt[:, 512:1024])
                mv = smp.tile([P, 2], F32, tag="mv")
                nc.vector.bn_aggr(mv[:], stats[:])
                sd = smp.tile([P, 1], F32, tag="sd")
                nc.scalar.activation(sd[:], mv[:, 1:2], AF.Sqrt, bias=epsp[:])
                rstd = smp.tile([P, 1], F32, tag="rstd")
                nc.vector.reciprocal(rstd[:], sd[:])
                mr = smp.tile([P, 1], F32, tag="mr")
                nc.vector.tensor_mul(mr[:], mv[:, 0:1], rstd[:])
                xn = xsp.tile([P, D], F32R, tag="xn", bufs=2)
                nc.vector.tensor_scalar(xn[:], xt[:], rstd[:], mr[:], OP.mult, OP.subtract)
                blk, col = i // 4, (i % 4) * P
                for c in range(DC):
                    if (blk, c) not in xnt:
                        xnt[(blk, c)] = xntp.tile([P, 512], F32R, tag="xnt", name=f"xnt_{blk}_{c}")
                    tp = ps_m.tile([P, P], F32R, tag="m")
                    nc.tensor.transpose(tp[:], xn[:, c * P:(c + 1) * P], ident[:])
                    nc.vector.tensor_copy(xnt[(blk, c)][:, col:col + P], tp[:])

            def qkv_block(blk):
                """q/k/v^T for token block blk from xnT chunks."""
                vtb = xntp.tile([P, 512], F32R, tag="vtb", bufs=2, name=f"vtb_{blk}")
                for grp, dst, col in ((0, qT, blk * 512), (1, kT, blk * 512), (2, vtb, 0)):
                    acc = ps_m.tile([P, 512], F32, tag="m")
                    nc.tensor.matmul(acc[:], bqkv[0:1, grp * P:(grp + 1) * P],
                                     ones512[:], start=True, stop=False)
                    for c in range(DC):
                        nc.tensor.matmul(acc[:], wqkv[c][:, grp * P:(grp + 1) * P],
                                         xnt[(blk, c)][:], start=False, stop=(c == DC - 1))
                    nc.vector.tensor_copy(dst[:, col:col + 512], acc[:])
                # free xnT chunks of this block
                for c in range(DC):
                    del xnt[(blk, c)]
                return vtb

            def vaug_block(blk, vtb):
                """transpose v^T block into v_aug layout (+ ones columns)."""
                for t in range(4):
                    i = blk * 4 + t   # token tile index
                    tp = ps_m.tile([P, P], F32R, tag="m")
                    nc.tensor.transpose(tp[:], vtb[:, t * P:(t + 1) * P], ident[:])
                    base = i * 130
                    nc.vector.tensor_copy(vaug[:, base:base + 64], tp[:, 0:64])
                    nc.vector.tensor_copy(vaug[:, base + 65:base + 129], tp[:, 64:128])
                    nc.vector.tensor_copy(vaug[:, base + 64:base + 65], onesp[:])
                    nc.vector.tensor_copy(vaug[:, base + 129:base + 130], onesp[:])

            for blk in range(NBLK):
                for t in range(4):
                    ln_tile(blk * 4 + t)
                vtb = qkv_block(blk)
                vaug_block(blk, vtb)

            # ---- phase 3: attention per (head, batch) --------------------
            def attention(h, b):
                """S^T -> exp -> PV (ones-augmented) -> normalize -> a2a_in."""
                hp = h * DH   # partition offset of this head in qT/kT
                for tqb in range(2):              # two 1024-wide tq blocks
                    q0 = b * N + tqb * 1024
                    sa = ps_sa.tile([65, 1024], F32, tag="sa")
                    for m in range(16):           # tk tiles of batch b
                        mt = b * 16 + m           # global token tile of tk
                        s = ps_s.tile([P, 1024], F32, tag="s")
                        for hf in range(2):
                            nc.tensor.matmul(
                                s[:, hf * 512:(hf + 1) * 512],
                                kT[hp:hp + DH, mt * P:(mt + 1) * P],
                                qT[hp:hp + DH, q0 + hf * 512:q0 + (hf + 1) * 512],
                                start=True, stop=True)
                        e = esp.tile([P, 1024], F32R, tag="e")
                        nc.scalar.activation(e[:], s[:], AF.Exp, bias=0.0, scale=SCALE)
                        for hf in range(2):
                            nc.tensor.matmul(
                                sa[:, hf * 512:(hf + 1) * 512],
                                vaug[:, mt * 130 + h * 65: mt * 130 + (h + 1) * 65],
                                e[:, hf * 512:(hf + 1) * 512],
                                start=(m == 0), stop=(m == 15))
                    for hf in range(2):
                        r = smp.tile([1, 512], F32R, tag="r", bufs=2)
                        with nc.allow_low_precision(reason="softmax denom in f32r"):
                            nc.vector.reciprocal(r[:], sa[64:65, hf * 512:(hf + 1) * 512])
                        rb = ps_m.tile([64, 512], F32, tag="m")
                        nc.tensor.matmul(rb[:], ones_col64[:], r[:], start=True, stop=True)
                        rb_sb = sanp.tile([DH, 512], F32, tag="rb_sb", bufs=2)
                        nc.vector.tensor_copy(rb_sb[:], rb[:])
                        saN = sanp.tile([DH, 512], F32, tag="saN")
                        nc.vector.tensor_mul(saN[:], sa[0:DH, hf * 512:(hf + 1) * 512], rb_sb[:])
                        j = b * 4 + tqb * 2 + hf  # flat 512-token block
                        nc.sync.dma_start(a2a_in[h][j, :, :], saN[:])

            for h in range(H_LOC):
                for b in range(B):
                    attention(h, b)
                nc.gpsimd.collective_compute(
                    "AllToAll", OP.bypass,
                    replica_groups=[[0, 1, 2, 3, 4, 5, 6, 7]],
                    ins=[a2a_in[h].opt()],
                    outs=[a2a_out[h].opt()],
                )

            # ---- phase 4: local out-projection ---------------------------
            xa = []
            for c in range(DC):
                t = xntp.tile([P, 512], F32R, tag="xnt")
                nc.gpsimd.dma_start(t[0:DH, :], a2a_out[0][c, :, :])
                nc.gpsimd.dma_start(t[DH:P, :], a2a_out[1][c, :, :])
                xa.append(t)
            for half in range(2):
                wo = []
                for c in range(DC):
                    t = xntp.tile([P, 512], F32R, tag="xnt", name=f"wout_{c}_{half}")
                    nc.gpsimd.dma_start(
                        t[:], wout_ext.ap()[c * P:(c + 1) * P, half * 512:(half + 1) * 512])
                    wo.append(t)
                for t in range(4):
                    acc = ps_m.tile([P, 512], F32, tag="m")
                    nc.tensor.matmul(acc[:], ones_col128[:],
                                     bout[0:1, half * 512:(half + 1) * 512],
                                     start=True, stop=False)
                    for c in range(DC):
                        nc.tensor.matmul(acc[:], xa[c][:, t * P:(t + 1) * P],
                                         wo[c][:],
                                         start=False, stop=(c == DC - 1))
                    ot = sanp.tile([P, 512], F32, tag="ot", bufs=2, name=f"ot_{t}_{half}")
                    nc.vector.tensor_copy(ot[:], acc[:])
                    nc.sync.dma_start(
                        out_ext.ap()[t * P:(t + 1) * P, half * 512:(half + 1) * 512],
                        ot[:])

    nc.compile()
    return nc


_NC_CACHE = None
_last_in_maps = None


def kernel(x, gamma, beta, w_qkv, w_out, b_out):
    global _NC_CACHE
    if _NC_CACHE is None:
        _NC_CACHE = _build()
    nc = _NC_CACHE

    x = np.ascontiguousarray(np.asarray(x, dtype=np.float32).reshape(NT, D))
    gamma = np.asarray(gamma, dtype=np.float32)
    beta = np.asarray(beta, dtype=np.float32)
    w_qkv = np.asarray(w_qkv, dtype=np.float32)
    w_out = np.ascontiguousarray(np.asarray(w_out, dtype=np.float32))
    b_out = np.asarray(b_out, dtype=np.float32)

    # fold LayerNorm's affine (gamma, beta) into the QKV projection
    w_eff = gamma[:, None] * w_qkv            # [1024, 3072]
    b_eff = beta @ w_qkv                      # [3072]
    ident = np.eye(P, dtype=np.float32)

    in_maps = []
    for g in range(8):
        cols = []
        for part in range(3):                 # q, k, v column slices of heads {2g, 2g+1}
            c0 = part * D + g * (H_LOC * DH)
            cols.append(np.arange(c0, c0 + H_LOC * DH))
        cols = np.concatenate(cols)
        in_maps.append({
            "x": x,
            "wqkv": np.ascontiguousarray(w_eff[:, cols]),
            "bqkv": np.ascontiguousarray(b_eff[cols][None, :]),
            "wout": w_out,
            "bout": np.ascontiguousarray(b_out[None, :]),
            "ident": ident,
        })

    global _last_in_maps
    _last_in_maps = in_maps
    res = run_bass_kernel_spmd(nc, in_maps, core_ids=list(range(8)))
    out = np.empty((NT, D), dtype=np.float32)
    for g in range(8):
        out[g * TOK_OUT:(g + 1) * TOK_OUT, :] = res.results[g]["out"]
    return out.reshape(B, N, D)
